# revision 2
# baseline (speedup 1.0000x reference)
"""Multi-head attention (causal, interleaved RoPE) on 8 TRN2 NeuronCores.

Sharding: core c = (batch b = c//4, head-group g = c%4). Each core computes
4 heads of one batch fully on-device (QKV proj + RoPE + causal attention +
partial Wo projection); host sums the 4 row-parallel Wo partials per batch.

Device layout (per core):
  qT/kT  [128, 2, 2048] f32r : partition p = head*32 + freq, free = (role, t)
         role 0 = even rope components (orig j = 2f), role 1 = odd (2f+1)
  v      [128, 16, 4, 65] f32r : partition = key t (mod 128), [V_h | ones]
  S^T    computed per (k-tile, q-tile) as K=32 x2(role) matmuls, 4 heads
         row-packed on the PE array via tile_position
  exp    ScalarE, scale=1/8 fused; causal: only needed column range computed,
         diagonal 128x128 block masked multiplicatively with a triu constant
  PV     lhsT=[V_h|1] stationary -> out^T [65, q] psum; row 64 = softmax sums
  norm   recip_approx_fast(sums) + gpsimd partition_broadcast + TT multiply
  Wo     outT [128, 2, 2048] f32r j-major chunks x wot -> partial [2048, 1024]
"""
import math

import numpy as np

import concourse.bass as bass
import concourse.mybir as mybir
import concourse.tile as tile
from concourse import bacc
from concourse.bass_utils import run_bass_kernel_spmd

f32 = mybir.dt.float32
f32r = mybir.dt.float32r
AF = mybir.ActivationFunctionType

T, D = 2048, 1024
G = 4            # heads per core
NTS = 4          # t-slices of 512
TS = T // NTS    # 512
KT = T // 128    # 16 key tiles
DCH = D // 128   # 8 contraction chunks
ROPE_BASE = 10000.0

_CACHE = {}


def _build():
    nc = bacc.Bacc(None, target_bir_lowering=False)
    xt = nc.dram_tensor("xt", [D, T], f32, kind="ExternalInput")
    wqt = nc.dram_tensor("wqt", [D, 256], f32, kind="ExternalInput")
    wkt = nc.dram_tensor("wkt", [D, 256], f32, kind="ExternalInput")
    wvt = nc.dram_tensor("wvt", [D, 256], f32, kind="ExternalInput")
    wot = nc.dram_tensor("wot", [256, D], f32, kind="ExternalInput")
    cosp = nc.dram_tensor("cosp", [128, T], f32, kind="ExternalInput")
    sinp = nc.dram_tensor("sinp", [128, T], f32, kind="ExternalInput")
    triu = nc.dram_tensor("triu", [128, 128], f32, kind="ExternalInput")
    vone = nc.dram_tensor("vone", [128, KT * G], f32, kind="ExternalInput")
    outp = nc.dram_tensor("outp", [T, D], f32, kind="ExternalOutput")

    xt_r = xt.rearrange("(dc p) t -> p dc t", p=128)
    wqt_r = wqt.rearrange("(dc p) j -> p dc j", p=128)
    wkt_r = wkt.rearrange("(dc p) j -> p dc j", p=128)
    wvt_r = wvt.rearrange("(dc p) j -> p dc j", p=128)
    wot_r = wot.rearrange("(c p) m -> p c m", p=128)
    outp_r = outp.rearrange("(tt p) m -> p tt m", p=128)

    with tile.TileContext(nc) as tc:
        with tc.tile_pool(name="const", bufs=1) as const:
            wq_sb = const.tile([128, DCH, 256], f32r)
            wk_sb = const.tile([128, DCH, 256], f32r)
            wv_sb = const.tile([128, DCH, 256], f32r)
            wo_sb = const.tile([128, 2, D], f32r)
            cos_sb = const.tile([128, T], f32)
            sin_sb = const.tile([128, T], f32)
            triu_sb = const.tile([128, 128], f32)
            qT = const.tile([128, 2, T], f32r)
            kT_ = const.tile([128, 2, T], f32r)
            v_sb = const.tile([128, KT, G, 65], f32r)
            outT = const.tile([128, 2, T], f32r)

            nc.sync.dma_start(wq_sb[:], wqt_r.bitcast(f32r))
            nc.sync.dma_start(wk_sb[:], wkt_r.bitcast(f32r))
            nc.sync.dma_start(wv_sb[:], wvt_r.bitcast(f32r))
            nc.sync.dma_start(wo_sb[:], wot_r.bitcast(f32r))
            nc.sync.dma_start(cos_sb[:], cosp[:])
            nc.sync.dma_start(sin_sb[:], sinp[:])
            nc.sync.dma_start(triu_sb[:], triu[:])
            nc.sync.dma_start(
                v_sb[:, :, :, 64:65],
                vone.rearrange("p (k g) -> p k g", g=G)[:, :, :, None].bitcast(f32r),
            )

            # ---- Phase 1: projections + RoPE ----
            with (
                tc.tile_pool(name="xtp", bufs=2) as xtp,
                tc.tile_pool(name="pps", bufs=2, space="PSUM") as pps,
                tc.tile_pool(name="ut", bufs=2) as ut,
                tc.tile_pool(name="vps", bufs=2, space="PSUM") as vps,
            ):
                for tsi in range(NTS):
                    sl = slice(tsi * TS, (tsi + 1) * TS)
                    xt_t = xtp.tile([128, DCH, TS], f32r, tag="xt")
                    nc.sync.dma_start(xt_t[:], xt_r[:, :, sl].bitcast(f32r))
                    for w_sb, dst in ((wq_sb, qT), (wk_sb, kT_)):
                        ps = pps.tile([128, 2, TS], f32, tag="p")
                        for role in (0, 1):
                            for d in range(DCH):
                                nc.tensor.matmul(
                                    ps[:, role, :],
                                    w_sb[:, d, role * 128:(role + 1) * 128],
                                    xt_t[:, d, :],
                                    start=(d == 0),
                                    stop=(d == DCH - 1),
                                )
                        uc = ut.tile([128, 2, TS], f32, tag="uc")
                        us = ut.tile([128, 2, TS], f32, tag="us")
                        nc.vector.tensor_mul(
                            uc[:],
                            ps[:],
                            cos_sb[:, None, sl].to_broadcast((128, 2, TS)),
                        )
                        nc.vector.tensor_mul(us[:, 0, :], ps[:, 1, :], sin_sb[:, sl])
                        nc.vector.tensor_mul(us[:, 1, :], ps[:, 0, :], sin_sb[:, sl])
                        nc.gpsimd.tensor_sub(dst[:, 0, sl], uc[:, 0, :], us[:, 0, :])
                        nc.gpsimd.tensor_add(dst[:, 1, sl], uc[:, 1, :], us[:, 1, :])
                    for st in range(4):
                        t0 = tsi * 4 + st
                        psv = vps.tile([128, 256], f32, tag="v")
                        for d in range(DCH):
                            nc.tensor.matmul(
                                psv[:],
                                xt_t[:, d, st * 128:(st + 1) * 128],
                                wv_sb[:, d, :],
                                start=(d == 0),
                                stop=(d == DCH - 1),
                            )
                        nc.scalar.copy(
                            v_sb[:, t0, :, 0:64],
                            psv[:].rearrange("p (g d) -> p g d", g=G),
                        )

            # ---- Phase 2: attention ----
            with (
                tc.tile_pool(name="sps", bufs=1, space="PSUM") as sps,
                tc.tile_pool(name="pvps", bufs=1, space="PSUM") as pvps,
                tc.tile_pool(name="expp", bufs=2) as expp,
                tc.tile_pool(name="nrm", bufs=2) as nrm,
            ):
                for qt in range(NTS):
                    pv = [pvps.tile([65, TS], f32, tag=f"pv{h}", name=f"pv{h}") for h in range(G)]
                    komax = 4 * qt + 3
                    for ko in range(komax + 1):
                        off = max(0, ko - 4 * qt) * 128
                        ps_s = sps.tile([128, G, TS], f32, tag="s")
                        for h in range(G):
                            for role in (0, 1):
                                nc.tensor.matmul(
                                    ps_s[:, h, off:],
                                    kT_[h * 32:(h + 1) * 32, role, ko * 128:(ko + 1) * 128],
                                    qT[h * 32:(h + 1) * 32, role, qt * TS + off:(qt + 1) * TS],
                                    start=(role == 0),
                                    stop=(role == 1),
                                    tile_position=(h * 32, 0),
                                )
                        ex = expp.tile([128, G, TS], f32r, tag="ex")
                        nc.scalar.activation(
                            ex[:, :, off:], ps_s[:, :, off:], AF.Exp, scale=0.125
                        )
                        if ko >= 4 * qt:
                            nc.vector.tensor_mul(
                                ex[:, :, off:off + 128],
                                ex[:, :, off:off + 128],
                                triu_sb[:, None, :].to_broadcast((128, G, 128)),
                            )
                        for h in range(G):
                            nc.tensor.matmul(
                                pv[h][:, off:],
                                v_sb[:, ko, h, :],
                                ex[:, h, off:],
                                start=(ko == 0),
                                stop=(ko == komax),
                            )
                    for h in range(G):
                        s0 = nrm.tile([1, TS], f32, tag="s0")
                        nc.vector.tensor_copy(s0[:], pv[h][64:65, :])
                        rc = nrm.tile([1, TS], f32, tag="rc")
                        nc.vector.reciprocal_approx_fast(out=rc[:], in_=s0[:])
                        rb = nrm.tile([64, TS], f32, tag="rb")
                        nc.gpsimd.partition_broadcast(rb[:], rc[:])
                        hc, hl = h // 2, h % 2
                        nc.vector.tensor_mul(
                            outT[hl * 64:(hl + 1) * 64, hc, qt * TS:(qt + 1) * TS],
                            pv[h][0:64, :],
                            rb[:],
                        )

            # ---- Phase 3: output projection (partial over this core's heads) ----
            with (
                tc.tile_pool(name="wops", bufs=2, space="PSUM") as wops,
                tc.tile_pool(name="osb", bufs=3) as osb,
            ):
                for tt in range(KT):
                    for mh in (0, 1):
                        po = wops.tile([128, TS], f32, tag="po")
                        for hc in (0, 1):
                            nc.tensor.matmul(
                                po[:],
                                outT[:, hc, tt * 128:(tt + 1) * 128],
                                wo_sb[:, hc, mh * TS:(mh + 1) * TS],
                                start=(hc == 0),
                                stop=(hc == 1),
                            )
                        ob = osb.tile([128, TS], f32, tag="ob")
                        nc.scalar.copy(ob[:], po[:])
                        nc.sync.dma_start(outp_r[:, tt, mh * TS:(mh + 1) * TS], ob[:])
    nc.compile()
    return nc


def _get_nc():
    if "nc" not in _CACHE:
        _CACHE["nc"] = _build()
    return _CACHE["nc"]


def _host_inputs(x, Wq, Wk, Wv, Wo):
    """Build per-core input dicts (host-side sharding / layout prep)."""
    jj = np.arange(256)
    role = jj // 128
    h = (jj % 128) // 32
    f = jj % 32
    inv_freq = (1.0 / (ROPE_BASE ** (np.arange(0, 64, 2, dtype=np.float64) / 64.0)))
    t = np.arange(T, dtype=np.float64)
    ang = t[None, :] * inv_freq[np.arange(128) % 32][:, None]   # [128, T]
    cosp = np.cos(ang).astype(np.float32)
    sinp = np.sin(ang).astype(np.float32)
    triu = (np.arange(128)[None, :] >= np.arange(128)[:, None]).astype(np.float32)
    vone = np.ones((128, KT * G), dtype=np.float32)

    in_maps = []
    for core in range(8):
        b, g = divmod(core, 4)
        jsel = (g * 4 + h) * 64 + 2 * f + role
        in_maps.append({
            "xt": np.ascontiguousarray(x[b].T),
            "wqt": np.ascontiguousarray(Wq[jsel, :].T),
            "wkt": np.ascontiguousarray(Wk[jsel, :].T),
            "wvt": np.ascontiguousarray(Wv[g * 256:(g + 1) * 256, :].T),
            "wot": np.ascontiguousarray(Wo[:, g * 256:(g + 1) * 256].T),
            "cosp": cosp,
            "sinp": sinp,
            "triu": triu,
            "vone": vone,
        })
    return in_maps


def run(x, Wq, Wk, Wv, Wo, trace=False):
    nc = _get_nc()
    in_maps = _host_inputs(x, Wq, Wk, Wv, Wo)
    res = run_bass_kernel_spmd(nc, in_maps, core_ids=list(range(8)), trace=trace)
    out = np.zeros((2, T, D), dtype=np.float64)
    for core in range(8):
        out[core // 4] += res.results[core]["outp"].astype(np.float64)
    return out.astype(np.float32), res


def kernel(x=None, mask=None, Wq=None, Wk=None, Wv=None, Wo=None, **_ignored):
    x = np.asarray(x, dtype=np.float32)
    Wq = np.asarray(Wq, dtype=np.float32)
    Wk = np.asarray(Wk, dtype=np.float32)
    Wv = np.asarray(Wv, dtype=np.float32)
    Wo = np.asarray(Wo, dtype=np.float32)
    out, _ = run(x, Wq, Wk, Wv, Wo, trace=False)
    return out


# revision 7
# speedup vs baseline: 1.1770x; 1.1770x over previous
"""Multi-head attention (causal, interleaved RoPE) on 8 TRN2 NeuronCores.

Sharding: core c = (batch b = c//4, head-group g = c%4). Each core computes
4 heads of one batch fully on-device (QKV proj + RoPE + causal attention +
partial Wo projection); host sums the 4 row-parallel Wo partials per batch.

Device layout (per core):
  qT/kT  [128, 2, 2048] f32r : partition p = head*32 + freq, free = (role, t)
         role 0 = even rope components (orig j = 2f), role 1 = odd (2f+1)
  v      [128, 16, 4, 65] f32r : partition = key t (mod 128), [V_h | ones]
  S^T    computed per (k-tile, q-tile) as K=32 x2(role) matmuls, 4 heads
         row-packed on the PE array via tile_position
  exp    ScalarE, scale=1/8 fused; causal: only needed column range computed,
         diagonal 128x128 block masked multiplicatively with a triu constant
  PV     lhsT=[V_h|1] stationary -> out^T [65, q] psum; row 64 = softmax sums
  norm   recip_approx_fast(sums) + gpsimd partition_broadcast + TT multiply
  Wo     outT [128, 2, 2048] f32r j-major chunks x wot -> partial [2048, 1024]
"""
import math

import numpy as np

import concourse.bass as bass
import concourse.mybir as mybir
import concourse.tile as tile
from concourse import bacc
from concourse.bass_utils import run_bass_kernel_spmd

f32 = mybir.dt.float32
f32r = mybir.dt.float32r
AF = mybir.ActivationFunctionType

T, D = 2048, 1024
G = 4            # heads per core
NTS = 4          # t-slices of 512
TS = T // NTS    # 512
KT = T // 128    # 16 key tiles
DCH = D // 128   # 8 contraction chunks
ROPE_BASE = 10000.0

_CACHE = {}


def _build():
    nc = bacc.Bacc(None, target_bir_lowering=False)
    xt = nc.dram_tensor("xt", [D, T], f32, kind="ExternalInput")
    wqt = nc.dram_tensor("wqt", [D, 256], f32, kind="ExternalInput")
    wkt = nc.dram_tensor("wkt", [D, 256], f32, kind="ExternalInput")
    wvt = nc.dram_tensor("wvt", [D, 256], f32, kind="ExternalInput")
    wot = nc.dram_tensor("wot", [256, D], f32, kind="ExternalInput")
    cosp = nc.dram_tensor("cosp", [128, T], f32, kind="ExternalInput")
    sinp = nc.dram_tensor("sinp", [128, T], f32, kind="ExternalInput")
    triu = nc.dram_tensor("triu", [128, 128], f32, kind="ExternalInput")
    outp = nc.dram_tensor("outp", [T, D], f32, kind="ExternalOutput")

    xt_r = xt.rearrange("(dc p) t -> p dc t", p=128)
    wqt_r = wqt.rearrange("(dc p) j -> p dc j", p=128)
    wkt_r = wkt.rearrange("(dc p) j -> p dc j", p=128)
    wvt_r = wvt.rearrange("(dc p) j -> p dc j", p=128)
    wot_r = wot.rearrange("(c p) m -> p c m", p=128)
    outp_r = outp.rearrange("(tt p) m -> p tt m", p=128)

    with tile.TileContext(nc) as tc:
        with tc.tile_pool(name="const", bufs=1) as const:
            wq_sb = const.tile([128, DCH, 256], f32r)
            wk_sb = const.tile([128, DCH, 256], f32r)
            wv_sb = const.tile([128, DCH, 256], f32r)
            wo_sb = const.tile([128, 2, D], f32r)
            cos_sb = const.tile([128, T], f32)
            sin_sb = const.tile([128, T], f32)
            triu_sb = const.tile([128, 128], f32)
            qT = const.tile([128, 2, T], f32r)
            kT_ = const.tile([128, 2, T], f32r)
            v_sb = const.tile([128, KT, G, 65], f32r)
            outT = const.tile([128, 2, T], f32r)

            nc.sync.dma_start(wq_sb[:], wqt_r.bitcast(f32r))
            nc.sync.dma_start(wk_sb[:], wkt_r.bitcast(f32r))
            nc.sync.dma_start(wv_sb[:], wvt_r.bitcast(f32r))
            nc.sync.dma_start(wo_sb[:], wot_r.bitcast(f32r))
            nc.sync.dma_start(cos_sb[:], cosp[:])
            nc.sync.dma_start(sin_sb[:], sinp[:])
            nc.sync.dma_start(triu_sb[:], triu[:])
            nc.vector.memset(v_sb[:, :, :, 64:65].bitcast(f32), 1.0)

            # ---- Phase 1: projections + RoPE ----
            with (
                tc.tile_pool(name="xtp", bufs=2) as xtp,
                tc.tile_pool(name="pps", bufs=3, space="PSUM") as pps,
                tc.tile_pool(name="ut", bufs=3) as ut,
                tc.tile_pool(name="vps", bufs=2, space="PSUM") as vps,
            ):
                for tsi in range(NTS):
                    sl = slice(tsi * TS, (tsi + 1) * TS)
                    xt_t = xtp.tile([128, DCH, TS], f32r, tag="xt")
                    nc.sync.dma_start(xt_t[:], xt_r[:, :, sl].bitcast(f32r))
                    for w_sb, dst in ((wq_sb, qT), (wk_sb, kT_)):
                        ps = pps.tile([128, 2, TS], f32, tag="p")
                        for role in (0, 1):
                            for d in range(DCH):
                                nc.tensor.matmul(
                                    ps[:, role, :],
                                    w_sb[:, d, role * 128:(role + 1) * 128],
                                    xt_t[:, d, :],
                                    start=(d == 0),
                                    stop=(d == DCH - 1),
                                )
                        uc = ut.tile([128, 2, TS], f32, tag="uc")
                        us = ut.tile([128, 2, TS], f32, tag="us")
                        nc.vector.tensor_mul(
                            uc[:],
                            ps[:],
                            cos_sb[:, None, sl].to_broadcast((128, 2, TS)),
                        )
                        nc.vector.tensor_mul(us[:, 0, :], ps[:, 1, :], sin_sb[:, sl])
                        nc.vector.tensor_mul(us[:, 1, :], ps[:, 0, :], sin_sb[:, sl])
                        nc.gpsimd.tensor_sub(dst[:, 0, sl], uc[:, 0, :], us[:, 0, :])
                        nc.gpsimd.tensor_add(dst[:, 1, sl], uc[:, 1, :], us[:, 1, :])
                    for st in range(4):
                        t0 = tsi * 4 + st
                        psv = vps.tile([128, 256], f32, tag="v")
                        for d in range(DCH):
                            nc.tensor.matmul(
                                psv[:],
                                xt_t[:, d, st * 128:(st + 1) * 128],
                                wv_sb[:, d, :],
                                start=(d == 0),
                                stop=(d == DCH - 1),
                            )
                        nc.scalar.copy(
                            v_sb[:, t0, :, 0:64],
                            psv[:].rearrange("p (g d) -> p g d", g=G),
                        )

            # ---- Phase 2: attention ----
            with (
                tc.tile_pool(name="sps", bufs=2, space="PSUM") as sps,
                tc.tile_pool(name="pvps", bufs=1, space="PSUM") as pvps,
                tc.tile_pool(name="expp", bufs=3) as expp,
                tc.tile_pool(name="nrm", bufs=2) as nrm,
            ):
                for qt in range(NTS):
                    pv = [pvps.tile([65, TS], f32, tag=f"pv{h}", name=f"pv{h}") for h in range(G)]
                    komax = 4 * qt + 3
                    for ko in range(komax + 1):
                        off = max(0, ko - 4 * qt) * 128
                        for pair in (0, 1):
                            ps_s = sps.tile([128, 2, TS], f32, tag="s", name="ps_s")
                            for hh in (0, 1):
                                h = 2 * pair + hh
                                for role in (0, 1):
                                    nc.tensor.matmul(
                                        ps_s[:, hh, off:],
                                        kT_[h * 32:(h + 1) * 32, role, ko * 128:(ko + 1) * 128],
                                        qT[h * 32:(h + 1) * 32, role, qt * TS + off:(qt + 1) * TS],
                                        start=(role == 0),
                                        stop=(role == 1),
                                        tile_position=(h * 32, 0),
                                    )
                            ex = expp.tile([128, 2, TS], f32r, tag="ex", name="ex")
                            nc.scalar.activation(
                                ex[:, :, off:], ps_s[:, :, off:], AF.Exp, scale=0.125
                            )
                            if ko >= 4 * qt:
                                nc.vector.tensor_mul(
                                    ex[:, :, off:off + 128],
                                    ex[:, :, off:off + 128],
                                    triu_sb[:, None, :].to_broadcast((128, 2, 128)),
                                )
                            for hh in (0, 1):
                                h = 2 * pair + hh
                                nc.tensor.matmul(
                                    pv[h][:, off:],
                                    v_sb[:, ko, h, :],
                                    ex[:, hh, off:],
                                    start=(ko == 0),
                                    stop=(ko == komax),
                                )
                    for h in range(G):
                        s0 = nrm.tile([1, TS], f32, tag="s0")
                        nc.vector.tensor_copy(s0[:], pv[h][64:65, :])
                        rc = nrm.tile([1, TS], f32, tag="rc")
                        nc.vector.reciprocal_approx_fast(out=rc[:], in_=s0[:])
                        rb = nrm.tile([64, TS], f32, tag="rb")
                        nc.gpsimd.partition_broadcast(rb[:], rc[:])
                        hc, hl = h // 2, h % 2
                        nc.vector.tensor_mul(
                            outT[hl * 64:(hl + 1) * 64, hc, qt * TS:(qt + 1) * TS],
                            pv[h][0:64, :],
                            rb[:],
                        )

            # ---- Phase 3: output projection (partial over this core's heads) ----
            with (
                tc.tile_pool(name="wops", bufs=2, space="PSUM") as wops,
                tc.tile_pool(name="osb", bufs=3) as osb,
            ):
                for tt in range(KT):
                    for mh in (0, 1):
                        po = wops.tile([128, TS], f32, tag="po")
                        for hc in (0, 1):
                            nc.tensor.matmul(
                                po[:],
                                outT[:, hc, tt * 128:(tt + 1) * 128],
                                wo_sb[:, hc, mh * TS:(mh + 1) * TS],
                                start=(hc == 0),
                                stop=(hc == 1),
                            )
                        ob = osb.tile([128, TS], f32, tag="ob")
                        nc.scalar.copy(ob[:], po[:])
                        nc.sync.dma_start(outp_r[:, tt, mh * TS:(mh + 1) * TS], ob[:])
    nc.compile()
    return nc


def _get_nc():
    if "nc" not in _CACHE:
        _CACHE["nc"] = _build()
    return _CACHE["nc"]


def _host_inputs(x, Wq, Wk, Wv, Wo):
    """Build per-core input dicts (host-side sharding / layout prep)."""
    jj = np.arange(256)
    role = jj // 128
    h = (jj % 128) // 32
    f = jj % 32
    inv_freq = (1.0 / (ROPE_BASE ** (np.arange(0, 64, 2, dtype=np.float64) / 64.0)))
    t = np.arange(T, dtype=np.float64)
    ang = t[None, :] * inv_freq[np.arange(128) % 32][:, None]   # [128, T]
    cosp = np.cos(ang).astype(np.float32)
    sinp = np.sin(ang).astype(np.float32)
    triu = (np.arange(128)[None, :] >= np.arange(128)[:, None]).astype(np.float32)

    in_maps = []
    for core in range(8):
        b, g = divmod(core, 4)
        jsel = (g * 4 + h) * 64 + 2 * f + role
        in_maps.append({
            "xt": np.ascontiguousarray(x[b].T),
            "wqt": np.ascontiguousarray(Wq[jsel, :].T),
            "wkt": np.ascontiguousarray(Wk[jsel, :].T),
            "wvt": np.ascontiguousarray(Wv[g * 256:(g + 1) * 256, :].T),
            "wot": np.ascontiguousarray(Wo[:, g * 256:(g + 1) * 256].T),
            "cosp": cosp,
            "sinp": sinp,
            "triu": triu,
        })
    return in_maps


def run(x, Wq, Wk, Wv, Wo, trace=False):
    nc = _get_nc()
    in_maps = _host_inputs(x, Wq, Wk, Wv, Wo)
    res = run_bass_kernel_spmd(nc, in_maps, core_ids=list(range(8)), trace=trace)
    out = np.zeros((2, T, D), dtype=np.float64)
    for core in range(8):
        out[core // 4] += res.results[core]["outp"].astype(np.float64)
    return out.astype(np.float32), res


def kernel(x=None, mask=None, Wq=None, Wk=None, Wv=None, Wo=None, **_ignored):
    x = np.asarray(x, dtype=np.float32)
    Wq = np.asarray(Wq, dtype=np.float32)
    Wk = np.asarray(Wk, dtype=np.float32)
    Wv = np.asarray(Wv, dtype=np.float32)
    Wo = np.asarray(Wo, dtype=np.float32)
    out, _ = run(x, Wq, Wk, Wv, Wo, trace=False)
    return out


# revision 10
# speedup vs baseline: 1.2799x; 1.0875x over previous
"""Multi-head attention (causal, interleaved RoPE) on 8 TRN2 NeuronCores.

Sharding: core c = (batch b = c//4, head-group g = c%4). Each core computes
4 heads of one batch fully on-device (QKV proj + RoPE + causal attention +
partial Wo projection); host sums the 4 row-parallel Wo partials per batch.

Device layout (per core):
  qT/kT  [128, 2, 2048] f32r : partition p = head*32 + freq, free = (role, t)
         role 0 = even rope components (orig j = 2f), role 1 = odd (2f+1)
  v      [128, 16, 4, 65] f32r : partition = key t (mod 128), [V_h | ones]
  S^T    computed per (k-tile, q-tile) as K=32 x2(role) matmuls, 4 heads
         row-packed on the PE array via tile_position
  exp    ScalarE, scale=1/8 fused; causal: only needed column range computed,
         diagonal 128x128 block masked multiplicatively with a triu constant
  PV     lhsT=[V_h|1] stationary -> out^T [65, q] psum; row 64 = softmax sums
  norm   recip_approx_fast(sums) + gpsimd partition_broadcast + TT multiply
  Wo     outT [128, 2, 2048] f32r j-major chunks x wot -> partial [2048, 1024]
"""
import math

import numpy as np

import concourse.bass as bass
import concourse.mybir as mybir
import concourse.tile as tile
from concourse import bacc
from concourse.bass_utils import run_bass_kernel_spmd

f32 = mybir.dt.float32
f32r = mybir.dt.float32r
AF = mybir.ActivationFunctionType

T, D = 2048, 1024
G = 4            # heads per core
NTS = 4          # t-slices of 512
TS = T // NTS    # 512
KT = T // 128    # 16 key tiles
DCH = D // 128   # 8 contraction chunks
ROPE_BASE = 10000.0

_CACHE = {}


def _build():
    nc = bacc.Bacc(None, target_bir_lowering=False)
    xt = nc.dram_tensor("xt", [D, T], f32, kind="ExternalInput")
    wqt = nc.dram_tensor("wqt", [D, 256], f32, kind="ExternalInput")
    wkt = nc.dram_tensor("wkt", [D, 256], f32, kind="ExternalInput")
    wvt = nc.dram_tensor("wvt", [D, 256], f32, kind="ExternalInput")
    wot = nc.dram_tensor("wot", [256, D], f32, kind="ExternalInput")
    cosp = nc.dram_tensor("cosp", [128, T], f32, kind="ExternalInput")
    sinp = nc.dram_tensor("sinp", [128, T], f32, kind="ExternalInput")
    triu = nc.dram_tensor("triu", [128, 128], f32, kind="ExternalInput")
    outp = nc.dram_tensor("outp", [T, D], f32, kind="ExternalOutput")

    xt_r = xt.rearrange("(dc p) t -> p dc t", p=128)
    wqt_r = wqt.rearrange("(dc p) j -> p dc j", p=128)
    wkt_r = wkt.rearrange("(dc p) j -> p dc j", p=128)
    wvt_r = wvt.rearrange("(dc p) j -> p dc j", p=128)
    wot_r = wot.rearrange("(c p) m -> p c m", p=128)
    outp_r = outp.rearrange("(tt p) m -> p tt m", p=128)

    with tile.TileContext(nc) as tc:
        with tc.tile_pool(name="const", bufs=1) as const:
            wq_sb = const.tile([128, DCH, 256], f32r)
            wk_sb = const.tile([128, DCH, 256], f32r)
            wv_sb = const.tile([128, DCH, 256], f32r)
            wo_sb = const.tile([128, 2, D], f32r)
            triu_sb = const.tile([128, 128], f32)
            qT = const.tile([128, 2, T], f32r)
            kTz0 = const.tile([128, 2, T], f32r)
            kTz1 = const.tile([128, 2, T], f32r)
            v_sb = const.tile([128, KT, G, 65], f32r)
            outT = const.tile([128, 2, T], f32r)

            nc.sync.dma_start(wq_sb[:], wqt_r.bitcast(f32r))
            nc.sync.dma_start(wk_sb[:], wkt_r.bitcast(f32r))
            nc.sync.dma_start(wv_sb[:], wvt_r.bitcast(f32r))
            nc.sync.dma_start(wo_sb[:], wot_r.bitcast(f32r))
            nc.sync.dma_start(triu_sb[:], triu[:])
            nc.vector.memset(v_sb[:, :, :, 64:65].bitcast(f32), 1.0)
            nc.vector.memset(kTz0[:].bitcast(f32), 0.0)
            nc.vector.memset(kTz1[:].bitcast(f32), 0.0)

            # ---- Phase 1: projections + RoPE ----
            with (
                tc.tile_pool(name="xtp", bufs=2) as xtp,
                tc.tile_pool(name="pps", bufs=3, space="PSUM") as pps,
                tc.tile_pool(name="ut", bufs=2) as ut,
                tc.tile_pool(name="vps", bufs=2, space="PSUM") as vps,
            ):
                cos_sb = xtp.tile([128, T], f32, tag="cos", name="cos_sb")
                sin_sb = xtp.tile([128, T], f32, tag="sin", name="sin_sb")
                nc.sync.dma_start(cos_sb[:], cosp[:])
                nc.sync.dma_start(sin_sb[:], sinp[:])
                for tsi in range(NTS):
                    sl = slice(tsi * TS, (tsi + 1) * TS)
                    xt_t = xtp.tile([128, DCH, TS], f32r, tag="xt")
                    nc.sync.dma_start(xt_t[:], xt_r[:, :, sl].bitcast(f32r))
                    for w_sb, dst in ((wq_sb, qT), (wk_sb, None)):
                        ps = pps.tile([128, 2, TS], f32, tag="p")
                        for role in (0, 1):
                            for d in range(DCH):
                                nc.tensor.matmul(
                                    ps[:, role, :],
                                    w_sb[:, d, role * 128:(role + 1) * 128],
                                    xt_t[:, d, :],
                                    start=(d == 0),
                                    stop=(d == DCH - 1),
                                )
                        uc = ut.tile([128, 2, TS], f32, tag="uc")
                        us = ut.tile([128, 2, TS], f32, tag="us")
                        nc.vector.tensor_mul(
                            uc[:],
                            ps[:],
                            cos_sb[:, None, sl].to_broadcast((128, 2, TS)),
                        )
                        nc.vector.tensor_mul(us[:, 0, :], ps[:, 1, :], sin_sb[:, sl])
                        nc.vector.tensor_mul(us[:, 1, :], ps[:, 0, :], sin_sb[:, sl])
                        # combine + relayout: row hh*64 + role*32 + f, chunk hp
                        # (head h = 2*hp + hh). Q goes to one tile; K to the
                        # hh-th zero-padded tile so S^T can use K=128 matmuls.
                        for h in range(G):
                            hp, hh = h // 2, h % 2
                            src = slice(h * 32, (h + 1) * 32)
                            d2 = dst if dst is qT else (kTz0, kTz1)[hh]
                            eng = nc.gpsimd if h % 2 == 0 else nc.vector
                            eng.tensor_sub(
                                d2[hh * 64:hh * 64 + 32, hp, sl],
                                uc[src, 0, :],
                                us[src, 0, :],
                            )
                            eng.tensor_add(
                                d2[hh * 64 + 32:(hh + 1) * 64, hp, sl],
                                uc[src, 1, :],
                                us[src, 1, :],
                            )
                    for st in range(4):
                        t0 = tsi * 4 + st
                        psv = vps.tile([128, 256], f32, tag="v")
                        for d in range(DCH):
                            nc.tensor.matmul(
                                psv[:],
                                xt_t[:, d, st * 128:(st + 1) * 128],
                                wv_sb[:, d, :],
                                start=(d == 0),
                                stop=(d == DCH - 1),
                            )
                        nc.scalar.copy(
                            v_sb[:, t0, :, 0:64],
                            psv[:].rearrange("p (g d) -> p g d", g=G),
                        )

            # ---- Phase 2: attention ----
            with (
                tc.tile_pool(name="sps", bufs=2, space="PSUM") as sps,
                tc.tile_pool(name="pvps", bufs=1, space="PSUM") as pvps,
                tc.tile_pool(name="expp", bufs=3) as expp,
                tc.tile_pool(name="nrm", bufs=2) as nrm,
            ):
                for qt in range(NTS):
                    pv = [pvps.tile([65, TS], f32, tag=f"pv{h}", name=f"pv{h}") for h in range(G)]
                    komax = 4 * qt + 3
                    for ko in range(komax + 1):
                        off = max(0, ko - 4 * qt) * 128
                        for pair in (0, 1):
                            ps_s = sps.tile([128, 2, TS], f32, tag="s", name="ps_s")
                            for hh in (0, 1):
                                nc.tensor.matmul(
                                    ps_s[:, hh, off:],
                                    (kTz0, kTz1)[hh][:, pair, ko * 128:(ko + 1) * 128],
                                    qT[:, pair, qt * TS + off:(qt + 1) * TS],
                                    start=True,
                                    stop=True,
                                )
                            ex = expp.tile([128, 2, TS], f32r, tag="ex", name="ex")
                            nc.scalar.activation(
                                ex[:, :, off:], ps_s[:, :, off:], AF.Exp, scale=0.125
                            )
                            if ko >= 4 * qt:
                                nc.vector.tensor_mul(
                                    ex[:, :, off:off + 128],
                                    ex[:, :, off:off + 128],
                                    triu_sb[:, None, :].to_broadcast((128, 2, 128)),
                                )
                            for hh in (0, 1):
                                h = 2 * pair + hh
                                nc.tensor.matmul(
                                    pv[h][:, off:],
                                    v_sb[:, ko, h, :],
                                    ex[:, hh, off:],
                                    start=(ko == 0),
                                    stop=(ko == komax),
                                )
                    for h in range(G):
                        s0 = nrm.tile([1, TS], f32, tag="s0")
                        nc.vector.tensor_copy(s0[:], pv[h][64:65, :])
                        rc = nrm.tile([1, TS], f32, tag="rc")
                        nc.vector.reciprocal_approx_fast(out=rc[:], in_=s0[:])
                        rb = nrm.tile([64, TS], f32, tag="rb")
                        nc.gpsimd.partition_broadcast(rb[:], rc[:])
                        hc, hl = h // 2, h % 2
                        nc.vector.tensor_mul(
                            outT[hl * 64:(hl + 1) * 64, hc, qt * TS:(qt + 1) * TS],
                            pv[h][0:64, :],
                            rb[:],
                        )

            # ---- Phase 3: output projection (partial over this core's heads) ----
            with (
                tc.tile_pool(name="wops", bufs=2, space="PSUM") as wops,
                tc.tile_pool(name="osb", bufs=3) as osb,
            ):
                for tt in range(KT):
                    for mh in (0, 1):
                        po = wops.tile([128, TS], f32, tag="po")
                        for hc in (0, 1):
                            nc.tensor.matmul(
                                po[:],
                                outT[:, hc, tt * 128:(tt + 1) * 128],
                                wo_sb[:, hc, mh * TS:(mh + 1) * TS],
                                start=(hc == 0),
                                stop=(hc == 1),
                            )
                        ob = osb.tile([128, TS], f32, tag="ob")
                        if (tt + mh) % 2 == 0:
                            nc.scalar.copy(ob[:], po[:])
                        else:
                            nc.vector.tensor_copy(ob[:], po[:])
                        nc.sync.dma_start(outp_r[:, tt, mh * TS:(mh + 1) * TS], ob[:])
    nc.compile()
    return nc


def _get_nc():
    if "nc" not in _CACHE:
        _CACHE["nc"] = _build()
    return _CACHE["nc"]


def _host_inputs(x, Wq, Wk, Wv, Wo):
    """Build per-core input dicts (host-side sharding / layout prep)."""
    jj = np.arange(256)
    role = jj // 128
    h = (jj % 128) // 32
    f = jj % 32
    inv_freq = (1.0 / (ROPE_BASE ** (np.arange(0, 64, 2, dtype=np.float64) / 64.0)))
    t = np.arange(T, dtype=np.float64)
    ang = t[None, :] * inv_freq[np.arange(128) % 32][:, None]   # [128, T]
    cosp = np.cos(ang).astype(np.float32)
    sinp = np.sin(ang).astype(np.float32)
    triu = (np.arange(128)[None, :] >= np.arange(128)[:, None]).astype(np.float32)

    in_maps = []
    for core in range(8):
        b, g = divmod(core, 4)
        jsel = (g * 4 + h) * 64 + 2 * f + role
        in_maps.append({
            "xt": np.ascontiguousarray(x[b].T),
            "wqt": np.ascontiguousarray(Wq[jsel, :].T),
            "wkt": np.ascontiguousarray(Wk[jsel, :].T),
            "wvt": np.ascontiguousarray(Wv[g * 256:(g + 1) * 256, :].T),
            "wot": np.ascontiguousarray(Wo[:, g * 256:(g + 1) * 256].T),
            "cosp": cosp,
            "sinp": sinp,
            "triu": triu,
        })
    return in_maps


def run(x, Wq, Wk, Wv, Wo, trace=False):
    nc = _get_nc()
    in_maps = _host_inputs(x, Wq, Wk, Wv, Wo)
    res = run_bass_kernel_spmd(nc, in_maps, core_ids=list(range(8)), trace=trace)
    out = np.zeros((2, T, D), dtype=np.float64)
    for core in range(8):
        out[core // 4] += res.results[core]["outp"].astype(np.float64)
    return out.astype(np.float32), res


def kernel(x=None, mask=None, Wq=None, Wk=None, Wv=None, Wo=None, **_ignored):
    x = np.asarray(x, dtype=np.float32)
    Wq = np.asarray(Wq, dtype=np.float32)
    Wk = np.asarray(Wk, dtype=np.float32)
    Wv = np.asarray(Wv, dtype=np.float32)
    Wo = np.asarray(Wo, dtype=np.float32)
    out, _ = run(x, Wq, Wk, Wv, Wo, trace=False)
    return out


# revision 11
# speedup vs baseline: 1.3402x; 1.0472x over previous
"""Multi-head attention (causal, interleaved RoPE) on 8 TRN2 NeuronCores.

Sharding: core c = (batch b = c//4, head-group g = c%4). Each core computes
4 heads of one batch fully on-device (QKV proj + RoPE + causal attention +
partial Wo projection); host sums the 4 row-parallel Wo partials per batch.

Device layout (per core):
  qT/kT  [128, 2, 2048] f32r : partition p = head*32 + freq, free = (role, t)
         role 0 = even rope components (orig j = 2f), role 1 = odd (2f+1)
  v      [128, 16, 4, 65] f32r : partition = key t (mod 128), [V_h | ones]
  S^T    computed per (k-tile, q-tile) as K=32 x2(role) matmuls, 4 heads
         row-packed on the PE array via tile_position
  exp    ScalarE, scale=1/8 fused; causal: only needed column range computed,
         diagonal 128x128 block masked multiplicatively with a triu constant
  PV     lhsT=[V_h|1] stationary -> out^T [65, q] psum; row 64 = softmax sums
  norm   recip_approx_fast(sums) + gpsimd partition_broadcast + TT multiply
  Wo     outT [128, 2, 2048] f32r j-major chunks x wot -> partial [2048, 1024]
"""
import math

import numpy as np

import concourse.bass as bass
import concourse.mybir as mybir
import concourse.tile as tile
from concourse import bacc
from concourse.bass_utils import run_bass_kernel_spmd

f32 = mybir.dt.float32
f32r = mybir.dt.float32r
bf16 = mybir.dt.bfloat16
AF = mybir.ActivationFunctionType

T, D = 2048, 1024
G = 4            # heads per core
NTS = 4          # t-slices of 512
TS = T // NTS    # 512
KT = T // 128    # 16 key tiles
DCH = D // 128   # 8 contraction chunks
ROPE_BASE = 10000.0

_CACHE = {}


def _build():
    nc = bacc.Bacc(None, target_bir_lowering=False)
    xt = nc.dram_tensor("xt", [D, T], f32, kind="ExternalInput")
    wqt = nc.dram_tensor("wqt", [D, 256], f32, kind="ExternalInput")
    wkt = nc.dram_tensor("wkt", [D, 256], f32, kind="ExternalInput")
    wvt = nc.dram_tensor("wvt", [D, 256], f32, kind="ExternalInput")
    wot = nc.dram_tensor("wot", [256, D], f32, kind="ExternalInput")
    cosp = nc.dram_tensor("cosp", [128, T], f32, kind="ExternalInput")
    sinp = nc.dram_tensor("sinp", [128, T], f32, kind="ExternalInput")
    triu = nc.dram_tensor("triu", [128, 128], f32, kind="ExternalInput")
    outp = nc.dram_tensor("outp", [T, D], f32, kind="ExternalOutput")

    xt_r = xt.rearrange("(dc p) t -> p dc t", p=128)
    wqt_r = wqt.rearrange("(dc p) j -> p dc j", p=128)
    wkt_r = wkt.rearrange("(dc p) j -> p dc j", p=128)
    wvt_r = wvt.rearrange("(dc p) j -> p dc j", p=128)
    wot_r = wot.rearrange("(c p) m -> p c m", p=128)
    outp_r = outp.rearrange("(tt p) m -> p tt m", p=128)

    with tile.TileContext(nc) as tc:
        with tc.tile_pool(name="const", bufs=1) as const:
            wq_sb = const.tile([128, DCH, 256], f32r)
            wk_sb = const.tile([128, DCH, 256], f32r)
            wv_sb = const.tile([128, DCH, 256], f32r)
            wo_sb = const.tile([128, 2, D], f32r)
            triu_sb = const.tile([128, 128], f32)
            qT = const.tile([128, 2, T], bf16)
            kTz0 = const.tile([128, 2, T], bf16)
            kTz1 = const.tile([128, 2, T], bf16)
            v_sb = const.tile([128, KT, G, 65], f32r)
            outT = const.tile([128, 2, T], f32r)

            nc.sync.dma_start(wq_sb[:], wqt_r.bitcast(f32r))
            nc.sync.dma_start(wk_sb[:], wkt_r.bitcast(f32r))
            nc.sync.dma_start(wv_sb[:], wvt_r.bitcast(f32r))
            nc.sync.dma_start(wo_sb[:], wot_r.bitcast(f32r))
            nc.sync.dma_start(triu_sb[:], triu[:])
            nc.vector.memset(v_sb[:, :, :, 64:65].bitcast(f32), 1.0)
            nc.vector.memset(kTz0[:], 0.0)
            nc.vector.memset(kTz1[:], 0.0)

            # ---- Phase 1: projections + RoPE ----
            with (
                tc.tile_pool(name="xtp", bufs=2) as xtp,
                tc.tile_pool(name="pps", bufs=3, space="PSUM") as pps,
                tc.tile_pool(name="ut", bufs=2) as ut,
                tc.tile_pool(name="vps", bufs=2, space="PSUM") as vps,
            ):
                cos_sb = xtp.tile([128, T], f32, tag="cos", name="cos_sb")
                sin_sb = xtp.tile([128, T], f32, tag="sin", name="sin_sb")
                nc.sync.dma_start(cos_sb[:], cosp[:])
                nc.sync.dma_start(sin_sb[:], sinp[:])
                for tsi in range(NTS):
                    sl = slice(tsi * TS, (tsi + 1) * TS)
                    xt_t = xtp.tile([128, DCH, TS], f32r, tag="xt")
                    nc.sync.dma_start(xt_t[:], xt_r[:, :, sl].bitcast(f32r))
                    for w_sb, dst in ((wq_sb, qT), (wk_sb, None)):
                        ps = pps.tile([128, 2, TS], f32, tag="p")
                        for role in (0, 1):
                            for d in range(DCH):
                                nc.tensor.matmul(
                                    ps[:, role, :],
                                    w_sb[:, d, role * 128:(role + 1) * 128],
                                    xt_t[:, d, :],
                                    start=(d == 0),
                                    stop=(d == DCH - 1),
                                )
                        uc = ut.tile([128, 2, TS], bf16, tag="uc")
                        us = ut.tile([128, 2, TS], bf16, tag="us")
                        nc.vector.tensor_mul(
                            uc[:],
                            ps[:],
                            cos_sb[:, None, sl].to_broadcast((128, 2, TS)),
                        )
                        nc.vector.tensor_mul(us[:, 0, :], ps[:, 1, :], sin_sb[:, sl])
                        nc.vector.tensor_mul(us[:, 1, :], ps[:, 0, :], sin_sb[:, sl])
                        # combine + relayout: row hh*64 + role*32 + f, chunk hp
                        # (head h = 2*hp + hh). Q goes to one tile; K to the
                        # hh-th zero-padded tile so S^T can use K=128 matmuls.
                        for h in range(G):
                            hp, hh = h // 2, h % 2
                            src = slice(h * 32, (h + 1) * 32)
                            d2 = dst if dst is qT else (kTz0, kTz1)[hh]
                            eng = nc.gpsimd if h % 2 == 0 else nc.vector
                            eng.tensor_sub(
                                d2[hh * 64:hh * 64 + 32, hp, sl],
                                uc[src, 0, :],
                                us[src, 0, :],
                            )
                            eng.tensor_add(
                                d2[hh * 64 + 32:(hh + 1) * 64, hp, sl],
                                uc[src, 1, :],
                                us[src, 1, :],
                            )
                    for st in range(4):
                        t0 = tsi * 4 + st
                        psv = vps.tile([128, 256], f32, tag="v")
                        for d in range(DCH):
                            nc.tensor.matmul(
                                psv[:],
                                xt_t[:, d, st * 128:(st + 1) * 128],
                                wv_sb[:, d, :],
                                start=(d == 0),
                                stop=(d == DCH - 1),
                            )
                        nc.scalar.copy(
                            v_sb[:, t0, :, 0:64],
                            psv[:].rearrange("p (g d) -> p g d", g=G),
                        )

            # ---- Phase 2: attention ----
            with (
                tc.tile_pool(name="sps", bufs=2, space="PSUM") as sps,
                tc.tile_pool(name="pvps", bufs=1, space="PSUM") as pvps,
                tc.tile_pool(name="expp", bufs=3) as expp,
                tc.tile_pool(name="nrm", bufs=2) as nrm,
            ):
                for qt in range(NTS):
                    pv = [pvps.tile([65, TS], f32, tag=f"pv{h}", name=f"pv{h}") for h in range(G)]
                    komax = 4 * qt + 3
                    for ko in range(komax + 1):
                        off = max(0, ko - 4 * qt) * 128
                        for pair in (0, 1):
                            ps_s = sps.tile([128, 2, TS], f32, tag="s", name="ps_s")
                            for hh in (0, 1):
                                nc.tensor.matmul(
                                    ps_s[:, hh, off:],
                                    (kTz0, kTz1)[hh][:, pair, ko * 128:(ko + 1) * 128],
                                    qT[:, pair, qt * TS + off:(qt + 1) * TS],
                                    start=True,
                                    stop=True,
                                )
                            ex = expp.tile([128, 2, TS], f32r, tag="ex", name="ex")
                            nc.scalar.activation(
                                ex[:, :, off:], ps_s[:, :, off:], AF.Exp, scale=0.125
                            )
                            if ko >= 4 * qt:
                                nc.vector.tensor_mul(
                                    ex[:, :, off:off + 128],
                                    ex[:, :, off:off + 128],
                                    triu_sb[:, None, :].to_broadcast((128, 2, 128)),
                                )
                            for hh in (0, 1):
                                h = 2 * pair + hh
                                nc.tensor.matmul(
                                    pv[h][:, off:],
                                    v_sb[:, ko, h, :],
                                    ex[:, hh, off:],
                                    start=(ko == 0),
                                    stop=(ko == komax),
                                )
                    for h in range(G):
                        s0 = nrm.tile([1, TS], f32, tag="s0")
                        nc.vector.tensor_copy(s0[:], pv[h][64:65, :])
                        rc = nrm.tile([1, TS], f32, tag="rc")
                        nc.vector.reciprocal_approx_fast(out=rc[:], in_=s0[:])
                        rb = nrm.tile([64, TS], f32, tag="rb")
                        nc.gpsimd.partition_broadcast(rb[:], rc[:])
                        hc, hl = h // 2, h % 2
                        nc.vector.tensor_mul(
                            outT[hl * 64:(hl + 1) * 64, hc, qt * TS:(qt + 1) * TS],
                            pv[h][0:64, :],
                            rb[:],
                        )

            # ---- Phase 3: output projection (partial over this core's heads) ----
            with (
                tc.tile_pool(name="wops", bufs=2, space="PSUM") as wops,
                tc.tile_pool(name="osb", bufs=3) as osb,
            ):
                for tt in range(KT):
                    for mh in (0, 1):
                        po = wops.tile([128, TS], f32, tag="po")
                        for hc in (0, 1):
                            nc.tensor.matmul(
                                po[:],
                                outT[:, hc, tt * 128:(tt + 1) * 128],
                                wo_sb[:, hc, mh * TS:(mh + 1) * TS],
                                start=(hc == 0),
                                stop=(hc == 1),
                            )
                        ob = osb.tile([128, TS], f32, tag="ob")
                        if (tt + mh) % 2 == 0:
                            nc.scalar.copy(ob[:], po[:])
                        else:
                            nc.vector.tensor_copy(ob[:], po[:])
                        nc.sync.dma_start(outp_r[:, tt, mh * TS:(mh + 1) * TS], ob[:])
    nc.compile()
    return nc


def _get_nc():
    if "nc" not in _CACHE:
        _CACHE["nc"] = _build()
    return _CACHE["nc"]


def _host_inputs(x, Wq, Wk, Wv, Wo):
    """Build per-core input dicts (host-side sharding / layout prep)."""
    jj = np.arange(256)
    role = jj // 128
    h = (jj % 128) // 32
    f = jj % 32
    inv_freq = (1.0 / (ROPE_BASE ** (np.arange(0, 64, 2, dtype=np.float64) / 64.0)))
    t = np.arange(T, dtype=np.float64)
    ang = t[None, :] * inv_freq[np.arange(128) % 32][:, None]   # [128, T]
    cosp = np.cos(ang).astype(np.float32)
    sinp = np.sin(ang).astype(np.float32)
    triu = (np.arange(128)[None, :] >= np.arange(128)[:, None]).astype(np.float32)

    in_maps = []
    for core in range(8):
        b, g = divmod(core, 4)
        jsel = (g * 4 + h) * 64 + 2 * f + role
        in_maps.append({
            "xt": np.ascontiguousarray(x[b].T),
            "wqt": np.ascontiguousarray(Wq[jsel, :].T),
            "wkt": np.ascontiguousarray(Wk[jsel, :].T),
            "wvt": np.ascontiguousarray(Wv[g * 256:(g + 1) * 256, :].T),
            "wot": np.ascontiguousarray(Wo[:, g * 256:(g + 1) * 256].T),
            "cosp": cosp,
            "sinp": sinp,
            "triu": triu,
        })
    return in_maps


def run(x, Wq, Wk, Wv, Wo, trace=False):
    nc = _get_nc()
    in_maps = _host_inputs(x, Wq, Wk, Wv, Wo)
    res = run_bass_kernel_spmd(nc, in_maps, core_ids=list(range(8)), trace=trace)
    out = np.zeros((2, T, D), dtype=np.float64)
    for core in range(8):
        out[core // 4] += res.results[core]["outp"].astype(np.float64)
    return out.astype(np.float32), res


def kernel(x=None, mask=None, Wq=None, Wk=None, Wv=None, Wo=None, **_ignored):
    x = np.asarray(x, dtype=np.float32)
    Wq = np.asarray(Wq, dtype=np.float32)
    Wk = np.asarray(Wk, dtype=np.float32)
    Wv = np.asarray(Wv, dtype=np.float32)
    Wo = np.asarray(Wo, dtype=np.float32)
    out, _ = run(x, Wq, Wk, Wv, Wo, trace=False)
    return out
